# revision 23
# baseline (speedup 1.0000x reference)
"""Mixtral sparse MoE block on 8 Trainium2 NeuronCores.

Expert-parallel: core e holds expert e's weights (w1/w3/w2 sharded on the E
axis), tokens are dispatched to cores by their top-2 expert assignment
(computed on host from the tiny replicated gate), each core runs the expert
GLU — y = (silu(x w1^T) * (x w3^T)) w2^T — over its token set in fp16 with
fp32 PSUM accumulation (measured ~5e-4 rel error; 16-bit operands stream
~6% faster than fp32r on the PE), the weighted combine is a host-side
scatter-add.

Device schedule, per core (fp16 PE floor is ~287us; the schedule exists to
keep the PE within a few us of it):
  Stage 1 keeps tokens in the matmul moving dim (512-token blocks) and
  produces actT [F, C] tiles in SBUF.  Stage 2 flips orientation: a
  128-token slice of actT is the stationary operand and w2^T columns stream
  at N=512, landing output directly in [C, H] layout.  F runs in two halves
  so the activation tensor fits in SBUF; half 0's stage-2 output is held in
  an SBUF fp32 accumulator and half 1 adds into it on the DVE, so a single
  fp16 output tensor leaves the core.

Startup path (the fp16 stream only sustains ~287us if the PE never waits):
  the startup is aggregate-DMA-bound (~2.75MB of x + first weights through
  a ~180GB/s fabric), so x is host-packed per-partition-contiguous and
  split in ko-pair pieces across all three DMA queues (sync/scalar/gpsimd)
  in PE need-order; f=0 accumulates PSUM in ko-pair sub-groups to match;
  dependency-free warmup matmuls cover the first DMA wait and predictable
  mid-f0 stalls, because the HAM clock gate drops the PE from 2.4GHz back
  to 1.2GHz after even a ~1.5us idle gap (with a ~5us re-ramp).
"""

import os

if os.environ.get("TRN_TERMINAL_POOL_IPS") and os.environ.get("JAX_PLATFORMS") == "cpu":
    # A cpu-pinned JAX would hide the axon-tunneled NeuronCores this kernel
    # runs on; the devices are reached via jax/PJRT, so let jax see them.
    os.environ.pop("JAX_PLATFORMS")

import numpy as np

import concourse.mybir as mybir
import concourse.tile as tile
from concourse import bacc
from concourse.bass_utils import run_bass_kernel_spmd

H = 1024
F = 3584
E = 8
TOP_K = 2
KO = H // 128     # 8   k-tiles over H (stage-1 contraction)
HB = H // 512     # 2   h-blocks (stage-2 moving dim)
FT = F // 128     # 28  f-tiles over F
N_HALVES = 2
FH = FT // N_HALVES  # 14 f-tiles per half
CB = 512          # token block (stage-1 moving-dim chunk)
C_CAP = 1024      # device token capacity; overflow beyond this is tiny and
                  # computed on host
N_WARMUP = 10     # PE warmup matmuls: cover the first x/w DMA + clock ramp
N_PAD = 12        # mid-f0 short-warmup pads per slow-arriving x piece

_nc_cache = {}


def _build(C):
    f16, f32 = mybir.dt.float16, mybir.dt.float32
    NB = C // CB   # 512-token blocks
    TT = C // 128  # token tiles for stage 2

    nc = bacc.Bacc("TRN2", target_bir_lowering=False, debug=False, num_devices=E)
    # x: host-packed [128, NB, KO, CB] so every block DMA is per-partition
    # contiguous (>=2KB descriptors -> full HBM bandwidth).
    xb = nc.dram_tensor("xb", [128, NB, KO, CB], f16, kind="ExternalInput")
    w1b = nc.dram_tensor("w1b", [FT, 128, KO, 128], f16, kind="ExternalInput")
    w3b = nc.dram_tensor("w3b", [FT, 128, KO, 128], f16, kind="ExternalInput")
    # w2: host-packed [128, FT, H] so a half's load is one contiguous
    # 28KB-per-partition read.
    w2b = nc.dram_tensor("w2b", [128, FT, H], f16, kind="ExternalInput")
    yb = nc.dram_tensor("yb", [C, H], f16, kind="ExternalOutput")

    with tile.TileContext(nc) as tc:
        with (
            tc.tile_pool(name="xpool", bufs=1) as xpool,
            tc.tile_pool(name="actpool", bufs=1) as actpool,
            tc.tile_pool(name="ypool", bufs=1) as ypool,
            tc.tile_pool(name="w13pool", bufs=4) as w13pool,
            tc.tile_pool(name="w2pool", bufs=1) as w2pool,
            tc.tile_pool(name="outpool", bufs=4) as outpool,
            tc.tile_pool(name="silupool", bufs=4) as silupool,
            tc.tile_pool(name="warmpool", bufs=1) as warmpool,
            tc.tile_pool(name="ps1", bufs=2, space="PSUM") as ps1,
            tc.tile_pool(name="ps2", bufs=4, space="PSUM") as ps2,
        ):
            # Warm tile memset leads the gpsimd queue (before its DMA) so the
            # PE warmup depends on nothing else.
            warm = warmpool.tile([128, 512], mybir.dt.bfloat16)
            nc.gpsimd.memset(warm[:], 0.0)

            # Startup is aggregate-DMA-bound: ~2.75MB (x + f=0/f=1 weights)
            # must land before the PE can stream, while the fabric delivers
            # only ~180GB/s total (sync ~62GB/s in the congested window and
            # ~150 after, gpsimd ~73, scalar ~44).  Gaps >4us also reset the
            # PE clock ramp.  So pieces are ko-pair sized and spread across
            # all three queues so arrival order tracks PE need order and any
            # stall stays short.
            w1t0 = w13pool.tile([128, KO, 128], f16, tag="w1t", name="w1t0")
            w3t0 = w13pool.tile([128, KO, 128], f16, tag="w3t", name="w3t0")
            xt = xpool.tile([128, NB, KO, CB], f16)
            # gpsimd: x b0 ko0-1 (first need), then x b1 ko0-3
            nc.gpsimd.dma_start(xt[:, 0, 0:2], xb[:, 0, 0:2])
            if NB > 1:
                nc.gpsimd.dma_start(xt[:, 1, 0:4], xb[:, 1, 0:4])
            # scalar: x b0 ko2-3, then x b1 ko4-5
            nc.scalar.dma_start(xt[:, 0, 2:4], xb[:, 0, 2:4])
            if NB > 1:
                nc.scalar.dma_start(xt[:, 1, 4:6], xb[:, 1, 4:6])
            # sync: f=0 weights in ko-pair pieces interleaved with the x b0
            # pieces the helpers don't carry, then x b1 ko6-7, then f>=1.
            nc.sync.dma_start(w1t0[:, 0:2], w1b[0][:, 0:2])
            nc.sync.dma_start(w3t0[:, 0:2], w3b[0][:, 0:2])
            nc.sync.dma_start(w1t0[:, 2:4], w1b[0][:, 2:4])
            nc.sync.dma_start(w3t0[:, 2:4], w3b[0][:, 2:4])
            nc.sync.dma_start(w1t0[:, 4:6], w1b[0][:, 4:6])
            nc.sync.dma_start(xt[:, 0, 4:6], xb[:, 0, 4:6])
            nc.sync.dma_start(w3t0[:, 4:6], w3b[0][:, 4:6])
            nc.sync.dma_start(w1t0[:, 6:8], w1b[0][:, 6:8])
            nc.sync.dma_start(xt[:, 0, 6:8], xb[:, 0, 6:8])
            nc.sync.dma_start(w3t0[:, 6:8], w3b[0][:, 6:8])
            if NB > 1:
                nc.sync.dma_start(xt[:, 1, 6:8], xb[:, 1, 6:8])
            for b in range(2, NB):
                nc.sync.dma_start(xt[:, b], xb[:, b])

            # Warm up the PE clock gate (HAM) during the initial DMA wait;
            # the real stream then starts near 2.4GHz instead of 1.2.  Also
            # used as padding inside f=0: a PE idle gap of even ~1.5us drops
            # the clock to 1.2GHz with a ~5us re-ramp, so predictable DMA
            # waits are filled with dependency-free matmuls instead.
            def _warm_mm(n, rows=512):
                for _ in range(n):
                    wp = ps2.tile([128, 512], f32, tag="py", name="wp")
                    nc.tensor.matmul(
                        wp[:, :rows], warm[:, :128], warm[:, :rows],
                        start=True, stop=True,
                    )

            _warm_mm(N_WARMUP)

            y0 = ypool.tile([128, TT, HB, 512], f32)  # half-0 stage-2 acc

            for half in range(N_HALVES):
                f0 = half * FH
                act = actpool.tile([128, FH, C], f16, tag="act")

                # Stage 1: actT[f, c] = silu(w1 xT) * (w3 xT), per 128-row
                # f tile, 512-token blocks in the moving dim.
                for fi in range(FH):
                    f = f0 + fi
                    if f == 0:
                        w1t, w3t = w1t0, w3t0
                    else:
                        w1t = w13pool.tile([128, KO, 128], f16, tag="w1t", name="w1t")
                        nc.sync.dma_start(w1t[:], w1b[f])
                        w3t = w13pool.tile([128, KO, 128], f16, tag="w3t", name="w3t")
                        nc.sync.dma_start(w3t[:], w3b[f])
                    for b in range(NB):
                        p1 = ps1.tile([128, CB], f32, tag="p1", name="p1")
                        p3 = ps1.tile([128, CB], f32, tag="p3", name="p3")
                        if f == 0 and b == 0:
                            # ko-pair sub-groups in DMA-arrival order, with
                            # warmup padding where the x pieces on the slow
                            # sync stream (ko4-5, ko6-7) land ~1.7us apart.
                            for g in range(KO // 2):
                                for p, wt in ((p1, w1t), (p3, w3t)):
                                    for ko in (2 * g, 2 * g + 1):
                                        nc.tensor.matmul(
                                            p, wt[:, ko], xt[:, b, ko],
                                            start=(ko == 0),
                                            stop=(ko == KO - 1),
                                        )
                                if g in (1, 2):
                                    # short pads: keep the HAM clock alive
                                    # through the sync-stream x waits at
                                    # minimal wasted PE time
                                    _warm_mm(N_PAD, rows=128)
                        else:
                            for ko in range(KO):
                                nc.tensor.matmul(
                                    p1, w1t[:, ko], xt[:, b, ko],
                                    start=(ko == 0), stop=(ko == KO - 1),
                                )
                            for ko in range(KO):
                                nc.tensor.matmul(
                                    p3, w3t[:, ko], xt[:, b, ko],
                                    start=(ko == 0), stop=(ko == KO - 1),
                                )
                        st = silupool.tile([128, CB], f32, tag="st", name="st")
                        nc.scalar.activation(
                            st, p1, mybir.ActivationFunctionType.Silu
                        )
                        nc.vector.tensor_tensor(
                            act[:, fi, b * CB : (b + 1) * CB], st, p3,
                            mybir.AluOpType.mult,
                        )

                # w2 for this half (issued after stage 1 so its DMA doesn't
                # delay the stage-1 weight stream; it overlaps with compute).
                w2t = w2pool.tile([128, FH, H], f16, tag="w2t")
                nc.sync.dma_start(w2t[:], w2b[:, f0 : f0 + FH])

                # Stage 2: y[tok, h] += actT[:, tok-tile].T @ w2T[:, h-block].
                # Half 0 parks in the fp32 SBUF accumulator; half 1 adds into
                # it on the DVE and ships one fp16 row-contiguous tensor.
                # All outputs ride sync (the fast queue, idle here): 256KB
                # every ~6us fits easily, and the final 128KB drains in <1us.
                out_queues = [nc.sync]
                for t in range(TT):
                    ts = slice(t * 128, (t + 1) * 128)
                    osb = None
                    if half == 1:
                        osb = outpool.tile([128, H], f16, tag="osb", name="osb")
                    for hb in range(HB):
                        hs = slice(hb * 512, (hb + 1) * 512)
                        py = ps2.tile([128, 512], f32, tag="py", name="py")
                        for kf in range(FH):
                            nc.tensor.matmul(
                                py, act[:, kf, ts], w2t[:, kf, hs],
                                start=(kf == 0), stop=(kf == FH - 1),
                            )
                        if half == 0:
                            nc.vector.tensor_copy(y0[:, t, hb], py[:])
                        elif t == TT - 1 and hb == HB - 1:
                            # Final tile: combine+store in 256-col pieces so
                            # the first piece's DMA overlaps the second's TT
                            # and the end-of-kernel drain waits only on the
                            # last 64KB — the TT+config+transfer chain after
                            # the final matmul shrinks from ~3.2us to ~1.8us.
                            for q in range(2):
                                qs = slice(hb * 512 + q * 256, hb * 512 + (q + 1) * 256)
                                nc.vector.tensor_tensor(
                                    osb[:, qs], py[:, q * 256 : (q + 1) * 256],
                                    y0[:, t, hb, q * 256 : (q + 1) * 256],
                                    mybir.AluOpType.add,
                                )
                                out_queues[0].dma_start(yb[ts, qs], osb[:, qs])
                        else:
                            nc.vector.tensor_tensor(
                                osb[:, hs], py[:], y0[:, t, hb],
                                mybir.AluOpType.add,
                            )
                            if t == TT - 1:
                                # earlier hb of the last tile ships per-hb so
                                # its transfer drains during the final group
                                out_queues[hb % len(out_queues)].dma_start(
                                    yb[ts, hs], osb[:, hs]
                                )
                    if half == 1 and t < TT - 1:
                        out_queues[t % len(out_queues)].dma_start(yb[ts], osb[:])
    nc.compile()
    return nc


def _routing(x, gate_w):
    """Replicates the reference router in fp32 numpy: softmax over expert
    logits, top-2, renormalized weights.  Verified to match jax bit-for-bit
    on expert selection for these inputs (min top2/top3 prob gap 3e-5)."""
    logits = x @ gate_w.T
    m = logits.max(-1, keepdims=True)
    p = np.exp(logits - m)
    p /= p.sum(-1, keepdims=True)
    top_i = np.argsort(-p, axis=-1, kind="stable")[:, :TOP_K]
    top_v = np.take_along_axis(p, top_i, axis=-1)
    top_v = top_v / top_v.sum(-1, keepdims=True)
    return top_i, top_v


def kernel(hidden_states, gate_w, w1, w3, w2):
    B, S, _ = hidden_states.shape
    x = np.ascontiguousarray(
        np.asarray(hidden_states, dtype=np.float32).reshape(-1, H)
    )
    gate_w = np.asarray(gate_w, dtype=np.float32)
    w1 = np.asarray(w1, dtype=np.float32)
    w3 = np.asarray(w3, dtype=np.float32)
    w2 = np.asarray(w2, dtype=np.float32)
    T = x.shape[0]

    top_i, top_v = _routing(x, gate_w)

    idx = [np.flatnonzero((top_i == e).any(axis=1)) for e in range(E)]
    wgt = []
    for e in range(E):
        sel = top_i[idx[e]] == e
        wgt.append(
            np.take_along_axis(top_v[idx[e]], np.argmax(sel, 1)[:, None], 1)[:, 0]
        )

    cmax = max(len(i) for i in idx)
    C = min(max(((cmax + CB - 1) // CB) * CB, CB), C_CAP)
    n_dev = [min(len(i), C) for i in idx]

    if C not in _nc_cache:
        _nc_cache[C] = _build(C)
    nc = _nc_cache[C]

    in_maps = []
    for e in range(E):
        x_pad = np.zeros((C, H), dtype=np.float32)
        x_pad[: n_dev[e]] = x[idx[e][: n_dev[e]]]
        # [C, H] -> [128p, NB, KO, CB]: block-contiguous per partition
        xb = np.ascontiguousarray(
            x_pad.reshape(C // CB, CB, KO, 128).transpose(3, 0, 2, 1)
        )
        t1 = w1[e].reshape(FT, 128, KO, 128)
        w1b = np.ascontiguousarray(t1.transpose(0, 3, 2, 1))
        t3 = w3[e].reshape(FT, 128, KO, 128)
        w3b = np.ascontiguousarray(t3.transpose(0, 3, 2, 1))
        # [H, F] -> [128p_f, FT, H]: half-contiguous per partition
        w2b = np.ascontiguousarray(w2[e].T.reshape(FT, 128, H).transpose(1, 0, 2))
        in_maps.append({
            "xb": xb.astype(np.float16),
            "w1b": w1b.astype(np.float16),
            "w3b": w3b.astype(np.float16),
            "w2b": w2b.astype(np.float16),
        })

    res = run_bass_kernel_spmd(nc, in_maps, core_ids=list(range(E)))

    out = np.zeros((T, H), dtype=np.float32)
    for e in range(E):
        y_e = res.results[e]["yb"].astype(np.float32)  # [C, H]
        out[idx[e][: n_dev[e]]] += wgt[e][: n_dev[e], None] * y_e[: n_dev[e]]
        if len(idx[e]) > n_dev[e]:
            # Overflow tokens past the capacity grid (a percent or so in the
            # worst-loaded expert): exact fp32 on host.
            xo = x[idx[e][n_dev[e] :]]
            h1 = xo @ w1[e].T
            a = (h1 / (1.0 + np.exp(-h1))) * (xo @ w3[e].T)
            yo = a @ w2[e].T
            out[idx[e][n_dev[e] :]] += wgt[e][n_dev[e] :, None] * yo
    return out.reshape(B, S, H)
